# revision 47
# baseline (speedup 1.0000x reference)
"""Trainium2 Bass kernel for nn_Net_5488968204310 (gnn_message_passing).

Single-head self-attention (D=128) over N=1024 nodes + gated residual update,
batch B=32, data-parallel across 8 NeuronCores (4 samples per core).

Design notes:
  - "T layout" (features d on partitions, nodes on free dim) for every matmul;
    all eight 128x128 weight matmuls contract over d.
  - x is uploaded twice: fp32 (final residual add) and bf16 (DMA-transposed
    straight into xT; layout transposes run on the DMA xbar, not the PE).
  - QK^T: logitsT chunks [m_chunk(128) x q(1024)] = kT_chunk.T @ qT; exp() on
    the scalar engine directly from PSUM with the 1/sqrt(D) scale folded in.
  - AV keeps v as the stationary operand (few LDWEIGHTS, dense 512-col
    streams): attnT[d,q] = sum_c v_nat[c].T @ expw[c]. The softmax denominator
    is a parallel ones.T @ expw accumulation; 1/denom = exp(-ln(denom)) on the
    scalar engine (DVE reciprocal is ~6 cyc/elem - too slow).
  - sigmoid(z+bg3) = exp(-ln(1+exp(-z-bg3))): every ACT function used (Exp,
    Ln, Relu-free paths) lives in the natural_log_exp_and_others table set,
    enforced by a scoped patch of the table metadata at compile time, so there
    is exactly one ACT_TABLE_LOAD per run.
  - Host folds: Wo1 -> Wo1 - I (so x@(Wo1-I)+msg = ret-x directly),
    Wo@Wg2 (gate path skips msg), bv -> bo terms, bias sums.
"""

import math

import numpy as np
import ml_dtypes

B, N, D = 32, 1024, 128
NCORES = 8
BPC = B // NCORES  # samples per core
NT = N // 128      # node chunks per sample

_CACHE = {}


def _bias_mode(vec):
    """(kind, value) where kind in {'zero', 'uniform', 'ap'}."""
    v = np.asarray(vec, np.float32)
    if not np.any(v):
        return ("zero", 0.0)
    if np.all(v == v.flat[0]):
        return ("uniform", float(v.flat[0]))
    return ("ap", 0.0)


def _build_nc(modes):
    import concourse.bacc as bacc
    import concourse.tile as tile
    from concourse import mybir
    from contextlib import ExitStack

    f32 = mybir.dt.float32
    bf16 = mybir.dt.bfloat16
    AF = mybir.ActivationFunctionType
    OP = mybir.AluOpType

    nc = bacc.Bacc("TRN2", target_bir_lowering=False, debug=False)

    x_d = nc.dram_tensor("x", [BPC, N, D], f32, kind="ExternalInput")
    xb_d = nc.dram_tensor("xbf", [BPC, N, D], bf16, kind="ExternalInput")
    out_d = nc.dram_tensor("out", [BPC, N, D], f32, kind="ExternalOutput")
    wnames = ["Wq", "Wk", "Wv", "Wo", "Wo1m", "Wg1", "Wog2", "Wg3"]
    w_d = {n: nc.dram_tensor(n, [D, D], bf16, kind="ExternalInput") for n in wnames}
    b_d = {
        n: nc.dram_tensor(n, [D, 1], f32, kind="ExternalInput")
        for n in modes if modes[n][0] == "ap"
    }

    s = 1.0 / math.sqrt(D)

    with tile.TileContext(nc) as tc, ExitStack() as ctx:
        consts = ctx.enter_context(tc.tile_pool(name="consts", bufs=1))
        sb = ctx.enter_context(tc.tile_pool(name="sb", bufs=2))
        sb3 = ctx.enter_context(tc.tile_pool(name="sb3", bufs=3))
        expp = ctx.enter_context(tc.tile_pool(name="expp", bufs=2))
        pw = ctx.enter_context(tc.tile_pool(name="pw", bufs=2, space="PSUM"))
        ph = ctx.enter_context(tc.tile_pool(name="ph", bufs=2, space="PSUM"))
        pav = ctx.enter_context(tc.tile_pool(name="pav", bufs=2, space="PSUM"))

        W = {}
        for n in wnames:
            t = consts.tile([D, D], bf16, tag=f"w_{n}")
            nc.sync.dma_start(t, w_d[n][:, :])
            W[n] = t
        BV = {}
        for n in b_d:
            t = consts.tile([D, 1], f32, tag=f"b_{n}")
            nc.sync.dma_start(t, b_d[n][:, :])
            BV[n] = t
        for n, (kind, val) in modes.items():
            if kind == "uniform":
                t = consts.tile([D, 1], f32, tag=f"b_{n}")
                nc.vector.memset(t, val)
                BV[n] = t

        def copyback(dst, src, bname, engine_copy):
            """psum->sbuf copy honoring the bias mode for `bname`."""
            kind, val = modes[bname]
            if kind == "zero":
                engine_copy(dst, src)
            else:
                nc.scalar.activation(dst, src, AF.Identity, bias=BV[bname])

        def act_bias(bname):
            kind, val = modes[bname]
            return 0.0 if kind == "zero" else BV[bname]

        ST = {}

        def phase1(b):
            """loads, q/k/v projections, QK^T + exp."""
            st = {}
            x_nat = sb3.tile([128, NT, D], f32, tag="x_nat")
            nc.sync.dma_start(x_nat, x_d[b].rearrange("(c p) d -> p c d", p=128))
            xT = sb3.tile([128, N], bf16, tag="xT")  # [d, n]
            nc.sync.dma_start_transpose(xT, xb_d[b])
            st["x_nat"], st["xT"] = x_nat, xT

            p_q = pw.tile([128, N], f32, tag="pw")
            nc.tensor.matmul(p_q[:, 0:512], W["Wq"], xT[:, 0:512], start=True, stop=True)
            nc.tensor.matmul(p_q[:, 512:1024], W["Wq"], xT[:, 512:1024], start=True, stop=True)
            qT = sb.tile([128, N], bf16, tag="qT")
            copyback(qT, p_q, "bq", nc.vector.tensor_copy)

            p_k = pw.tile([128, N], f32, tag="pw")
            nc.tensor.matmul(p_k[:, 0:512], W["Wk"], xT[:, 0:512], start=True, stop=True)
            nc.tensor.matmul(p_k[:, 512:1024], W["Wk"], xT[:, 512:1024], start=True, stop=True)
            kT = sb.tile([128, N], bf16, tag="kT")
            copyback(kT, p_k, "bk", nc.vector.tensor_copy)

            p_v = pw.tile([128, N], f32, tag="pw")
            nc.tensor.matmul(p_v[:, 0:512], W["Wv"], xT[:, 0:512], start=True, stop=True)
            nc.tensor.matmul(p_v[:, 512:1024], W["Wv"], xT[:, 512:1024], start=True, stop=True)
            vT = sb.tile([128, N], bf16, tag="vT")
            nc.vector.tensor_copy(vT, p_v)
            v_nat = sb.tile([128, NT, 128], bf16, tag="v_nat")
            nc.sync.dma_start_transpose(v_nat, vT)
            st["v_nat"] = v_nat

            # QK^T in natural orientation: logits chunk [q_chunk(128) x m(1024)].
            # exp's accum_out gives the softmax denominator per-partition for
            # free; gpsimd normalize_recip applies 1/denom; the normalized
            # weights are DMA-transposed into the [m, q] layout AV needs.
            expwT = expp.tile([128, NT, NT, 128], bf16, tag="expwT")  # [m', cm, cq, q']
            den = sb.tile([128, NT], f32, tag="den")
            for c in range(NT):
                p_l = pw.tile([128, N], f32, tag="pw")
                qTc = qT[:, c * 128:(c + 1) * 128]
                nc.tensor.matmul(p_l[:, 0:512], qTc, kT[:, 0:512], start=True, stop=True)
                nc.tensor.matmul(p_l[:, 512:1024], qTc, kT[:, 512:1024], start=True, stop=True)
                ew32 = sb.tile([128, N], f32, tag="ew32")
                nc.scalar.activation(ew32, p_l, AF.Exp, scale=s, accum_out=den[:, c:c + 1])
                ewbf = sb.tile([128, N], bf16, tag="ewbf")
                nc.gpsimd.normalize_recip(ewbf, ew32, den[:, c:c + 1])
                nc.sync.dma_start_transpose(expwT[:, :, c, :], ewbf)
            st["expwT"] = expwT
            return st

        def phase2(st):
            """AV: attnT[d, q] = sum_cm v_nat[cm].T @ expwT[cm] (pre-normalized)."""
            expwT, v_nat = st["expwT"], st["v_nat"]
            attnT = sb.tile([128, N], bf16, tag="attnT")
            H = NT // 2
            for h in range(2):
                sl = slice(h * 512, (h + 1) * 512)
                p_av = pav.tile([128, 512], f32, tag="pav")
                for c in range(NT):
                    nc.tensor.matmul(
                        p_av, v_nat[:, c, :], expwT[:, c, h * H:(h + 1) * H, :],
                        start=(c == 0), stop=(c == NT - 1),
                    )
                nc.vector.tensor_copy(attnT[:, sl], p_av)
            st["attnT"] = attnT

        def phase3(b, st):
            """gated update tail, pipelined in 512-wide halves; store."""
            x_nat, xT, attnT = st["x_nat"], st["xT"], st["attnT"]
            u = sb.tile([128, N], f32, tag="u")
            gp = sb.tile([128, N], bf16, tag="gp")
            sgy = sb.tile([128, N], f32, tag="sgy")
            sgw = sb.tile([128, N], f32, tag="sgw")
            gate = sb.tile([128, N], f32, tag="gate")
            dlt = sb.tile([128, N], bf16, tag="dlt")
            dlt_nat = sb.tile([128, NT, 128], bf16, tag="dlt_nat")
            o = sb.tile([128, NT, D], f32, tag="o")
            out_r = out_d[b].rearrange("(c p) d -> p c d", p=128)
            H = NT // 2
            for h in range(2):
                sl = slice(h * 512, (h + 1) * 512)
                cs = slice(h * H, (h + 1) * H)

                p_m = ph.tile([128, 512], f32, tag="pwh")
                nc.tensor.matmul(p_m, W["Wo"], attnT[:, sl], start=True, stop=False)
                nc.tensor.matmul(p_m, W["Wo1m"], xT[:, sl], start=False, stop=True)
                copyback(u[:, sl], p_m, "bo_u", nc.vector.tensor_copy)

                p_g = ph.tile([128, 512], f32, tag="pwh")
                nc.tensor.matmul(p_g, W["Wg1"], xT[:, sl], start=True, stop=False)
                nc.tensor.matmul(p_g, W["Wog2"], attnT[:, sl], start=False, stop=True)
                if modes["bo_g"][0] == "zero":
                    nc.vector.tensor_scalar(gp[:, sl], p_g, 0.0, None, op0=OP.max)
                else:
                    nc.scalar.activation(gp[:, sl], p_g, AF.Relu, bias=act_bias("bo_g"))

                p_g3 = ph.tile([128, 512], f32, tag="pwh")
                nc.tensor.matmul(p_g3, W["Wg3"], gp[:, sl], start=True, stop=True)
                nc.scalar.activation(sgy[:, sl], p_g3, AF.Exp, scale=-1.0, bias=act_bias("bg3n"))
                nc.scalar.activation(sgw[:, sl], sgy[:, sl], AF.Ln, bias=1.0)
                nc.scalar.activation(gate[:, sl], sgw[:, sl], AF.Exp, scale=-1.0)

                nc.vector.tensor_mul(dlt[:, sl], u[:, sl], gate[:, sl])
                nc.sync.dma_start_transpose(dlt_nat[:, cs, :], dlt[:, sl])
                nc.vector.tensor_add(o[:, cs, :], dlt_nat[:, cs, :], x_nat[:, cs, :])
                nc.sync.dma_start(out_r[:, cs, :], o[:, cs, :])

        # Software pipeline: emit P3(k-2), P2(k-1), P1(k) per step so each
        # engine's in-order stream interleaves three samples.
        for k in range(BPC + 2):
            if 0 <= k - 2:
                phase3(k - 2, ST[k - 2])
            if 0 <= k - 1 < BPC:
                phase2(ST[k - 1])
            if k < BPC:
                ST[k] = phase1(k)

    # Force Exp and Ln to resolve to the one table set that holds both
    # (natural_log_exp_and_others): contents-only lie to the set chooser,
    # dict order (= act_func_set_id) preserved; the set actually loaded at
    # runtime does contain both functions.
    import concourse.bacc as bacc_mod

    real_get = bacc_mod.get_activation_tables
    target = "natural_log_exp_and_others"

    def patched_get(arch):
        tabs = real_get(arch)
        strip = {AF.Exp, AF.Ln}
        return {
            name: (set(fns) if name == target else set(fns) - strip)
            for name, fns in tabs.items()
        }

    bacc_mod.get_activation_tables = patched_get
    try:
        nc.compile()
    finally:
        bacc_mod.get_activation_tables = real_get
    return nc


def _prep_host(inputs):
    """Host-side: fold weights/biases; returns (f32 inputs, weights bf16, biases)."""
    f32 = np.float32
    g = {k: np.asarray(v, f32) for k, v in inputs.items()}

    Wo1m = g["Wo1"] - np.eye(D, dtype=f32)
    Wog2 = g["Wo"] @ g["Wg2"]                      # msg path folded into gate
    bo_msg = g["bo"] + g["bv"] @ g["Wo"]           # bv folded through Wo
    bo_u = bo_msg + g["bo1"]                       # msg bias + ret bias
    bo_g = bo_msg @ g["Wg2"] + g["bg1"] + g["bg2"]
    bg3n = -g["bg3"]

    wmap = {
        "Wq": g["Wq"], "Wk": g["Wk"], "Wv": g["Wv"], "Wo": g["Wo"],
        "Wo1m": Wo1m, "Wg1": g["Wg1"], "Wog2": Wog2, "Wg3": g["Wg3"],
    }
    bmap = {
        "bq": g["bq"], "bk": g["bk"],
        "bo_u": bo_u, "bo_g": bo_g, "bg3n": bg3n,
    }
    bf16 = ml_dtypes.bfloat16
    wcast = {n: np.ascontiguousarray(w.astype(bf16)) for n, w in wmap.items()}
    return g, wcast, bmap


def _prep_inputs(inputs):
    g, wcast, bmap = _prep_host(inputs)
    modes = {n: _bias_mode(v) for n, v in bmap.items()}
    base = dict(wcast)
    for n, v in bmap.items():
        if modes[n][0] == "ap":
            base[n] = np.ascontiguousarray(v.reshape(D, 1).astype(np.float32))
    x = np.ascontiguousarray(g["x"])
    xbf = np.ascontiguousarray(x.astype(ml_dtypes.bfloat16))
    in_maps = []
    for c in range(NCORES):
        m = dict(base)
        m["x"] = np.ascontiguousarray(x[c * BPC:(c + 1) * BPC])
        m["xbf"] = np.ascontiguousarray(xbf[c * BPC:(c + 1) * BPC])
        in_maps.append(m)
    return in_maps, modes


def kernel(**inputs):
    from concourse.bass_utils import run_bass_kernel_spmd

    in_maps, modes = _prep_inputs(inputs)
    key = tuple(sorted((n, k[0], k[1]) for n, k in modes.items()))
    if _CACHE.get("key") != key:
        _CACHE["nc"] = _build_nc(modes)
        _CACHE["key"] = key
    nc = _CACHE["nc"]

    res = run_bass_kernel_spmd(nc, in_maps, list(range(NCORES)))
    out = np.concatenate([r["out"] for r in res.results], axis=0)
    return out.astype(np.float32)


# revision 48
# speedup vs baseline: 1.3716x; 1.3716x over previous
"""Trainium2 Bass kernel for nn_Net_5488968204310 (gnn_message_passing).

Single-head self-attention (D=128) over N=1024 nodes + gated residual update,
batch B=32, data-parallel across 8 NeuronCores (4 samples per core).

Design notes:
  - "T layout" (features d on partitions, nodes on free dim) for every matmul;
    all eight 128x128 weight matmuls contract over d.
  - x is uploaded twice: fp32 (final residual add) and bf16 (DMA-transposed
    straight into xT; layout transposes run on the DMA xbar, not the PE).
  - QK^T: logitsT chunks [m_chunk(128) x q(1024)] = kT_chunk.T @ qT; exp() on
    the scalar engine directly from PSUM with the 1/sqrt(D) scale folded in.
  - AV keeps v as the stationary operand (few LDWEIGHTS, dense 512-col
    streams): attnT[d,q] = sum_c v_nat[c].T @ expw[c]. The softmax denominator
    is a parallel ones.T @ expw accumulation; 1/denom = exp(-ln(denom)) on the
    scalar engine (DVE reciprocal is ~6 cyc/elem - too slow).
  - sigmoid(z+bg3) = exp(-ln(1+exp(-z-bg3))): every ACT function used (Exp,
    Ln, Relu-free paths) lives in the natural_log_exp_and_others table set,
    enforced by a scoped patch of the table metadata at compile time, so there
    is exactly one ACT_TABLE_LOAD per run.
  - Host folds: Wo1 -> Wo1 - I (so x@(Wo1-I)+msg = ret-x directly),
    Wo@Wg2 (gate path skips msg), bv -> bo terms, bias sums.
"""

import math

import numpy as np
import ml_dtypes

B, N, D = 32, 1024, 128
NCORES = 8
BPC = B // NCORES  # samples per core
NT = N // 128      # node chunks per sample

_CACHE = {}


def _bias_mode(vec):
    """(kind, value) where kind in {'zero', 'uniform', 'ap'}."""
    v = np.asarray(vec, np.float32)
    if not np.any(v):
        return ("zero", 0.0)
    if np.all(v == v.flat[0]):
        return ("uniform", float(v.flat[0]))
    return ("ap", 0.0)


def _build_nc(modes):
    import concourse.bacc as bacc
    import concourse.tile as tile
    from concourse import mybir
    from contextlib import ExitStack

    f32 = mybir.dt.float32
    bf16 = mybir.dt.bfloat16
    AF = mybir.ActivationFunctionType
    OP = mybir.AluOpType

    nc = bacc.Bacc("TRN2", target_bir_lowering=False, debug=False)

    x_d = nc.dram_tensor("x", [BPC, N, D], f32, kind="ExternalInput")
    xb_d = nc.dram_tensor("xbf", [BPC, N, D], bf16, kind="ExternalInput")
    out_d = nc.dram_tensor("out", [BPC, N, D], f32, kind="ExternalOutput")
    wnames = ["Wq", "Wk", "Wv", "Wo", "Wo1m", "Wg1", "Wog2", "Wg3"]
    w_d = {n: nc.dram_tensor(n, [D, D], bf16, kind="ExternalInput") for n in wnames}
    b_d = {
        n: nc.dram_tensor(n, [D, 1], f32, kind="ExternalInput")
        for n in modes if modes[n][0] == "ap"
    }

    s = 1.0 / math.sqrt(D)

    with tile.TileContext(nc) as tc, ExitStack() as ctx:
        consts = ctx.enter_context(tc.tile_pool(name="consts", bufs=1))
        sb = ctx.enter_context(tc.tile_pool(name="sb", bufs=2))
        sb3 = ctx.enter_context(tc.tile_pool(name="sb3", bufs=3))
        expp = ctx.enter_context(tc.tile_pool(name="expp", bufs=2))
        pw = ctx.enter_context(tc.tile_pool(name="pw", bufs=2, space="PSUM"))
        ph = ctx.enter_context(tc.tile_pool(name="ph", bufs=2, space="PSUM"))
        pav = ctx.enter_context(tc.tile_pool(name="pav", bufs=1, space="PSUM"))
        pden = ctx.enter_context(tc.tile_pool(name="pden", bufs=1, space="PSUM"))

        W = {}
        for n in wnames:
            t = consts.tile([D, D], bf16, tag=f"w_{n}")
            nc.sync.dma_start(t, w_d[n][:, :])
            W[n] = t
        ones = consts.tile([128, 128], bf16, tag="ones")
        nc.vector.memset(ones, 1.0)
        BV = {}
        for n in b_d:
            t = consts.tile([D, 1], f32, tag=f"b_{n}")
            nc.sync.dma_start(t, b_d[n][:, :])
            BV[n] = t
        for n, (kind, val) in modes.items():
            if kind == "uniform":
                t = consts.tile([D, 1], f32, tag=f"b_{n}")
                nc.vector.memset(t, val)
                BV[n] = t

        def copyback(dst, src, bname, engine_copy):
            """psum->sbuf copy honoring the bias mode for `bname`."""
            kind, val = modes[bname]
            if kind == "zero":
                engine_copy(dst, src)
            else:
                nc.scalar.activation(dst, src, AF.Identity, bias=BV[bname])

        def act_bias(bname):
            kind, val = modes[bname]
            return 0.0 if kind == "zero" else BV[bname]

        ST = {}

        def phase1(b):
            """loads, q/k/v projections, QK^T + exp."""
            st = {}
            x_nat = sb3.tile([128, NT, D], f32, tag="x_nat")
            nc.sync.dma_start(x_nat, x_d[b].rearrange("(c p) d -> p c d", p=128))
            xT = sb3.tile([128, N], bf16, tag="xT")  # [d, n]
            nc.sync.dma_start_transpose(xT, xb_d[b])
            st["x_nat"], st["xT"] = x_nat, xT

            p_q = pw.tile([128, N], f32, tag="pw")
            nc.tensor.matmul(p_q[:, 0:512], W["Wq"], xT[:, 0:512], start=True, stop=True)
            nc.tensor.matmul(p_q[:, 512:1024], W["Wq"], xT[:, 512:1024], start=True, stop=True)
            qT = sb.tile([128, N], bf16, tag="qT")
            copyback(qT, p_q, "bq", nc.vector.tensor_copy)

            p_k = pw.tile([128, N], f32, tag="pw")
            nc.tensor.matmul(p_k[:, 0:512], W["Wk"], xT[:, 0:512], start=True, stop=True)
            nc.tensor.matmul(p_k[:, 512:1024], W["Wk"], xT[:, 512:1024], start=True, stop=True)
            kT = sb.tile([128, N], bf16, tag="kT")
            copyback(kT, p_k, "bk", nc.vector.tensor_copy)

            p_v = pw.tile([128, N], f32, tag="pw")
            nc.tensor.matmul(p_v[:, 0:512], W["Wv"], xT[:, 0:512], start=True, stop=True)
            nc.tensor.matmul(p_v[:, 512:1024], W["Wv"], xT[:, 512:1024], start=True, stop=True)
            vT = sb.tile([128, N], bf16, tag="vT")
            nc.vector.tensor_copy(vT, p_v)
            v_nat = sb.tile([128, NT, 128], bf16, tag="v_nat")
            nc.sync.dma_start_transpose(v_nat, vT)
            st["v_nat"] = v_nat

            expw = expp.tile([128, NT, N], bf16, tag="expw")  # [m', c_m, q]
            for c in range(NT):
                p_l = pw.tile([128, N], f32, tag="pw")
                kTc = kT[:, c * 128:(c + 1) * 128]
                nc.tensor.matmul(p_l[:, 0:512], kTc, qT[:, 0:512], start=True, stop=True)
                nc.tensor.matmul(p_l[:, 512:1024], kTc, qT[:, 512:1024], start=True, stop=True)
                nc.scalar.activation(expw[:, c, :], p_l, AF.Exp, scale=s)
            st["expw"] = expw
            return st

        def phase2(st):
            """denominator + reciprocal + AV + normalize -> attnT."""
            expw, v_nat = st["expw"], st["v_nat"]
            lnd = sb.tile([128, N], f32, tag="lnd")
            rb = sb.tile([128, N], f32, tag="rb")
            attnT = sb.tile([128, N], bf16, tag="attnT")
            for h in range(2):
                sl = slice(h * 512, (h + 1) * 512)
                p_dn = pden.tile([128, 512], f32, tag="pden")
                for c in range(NT):
                    nc.tensor.matmul(p_dn, ones, expw[:, c, sl], start=(c == 0), stop=(c == NT - 1))
                nc.scalar.activation(lnd[:, sl], p_dn, AF.Ln)
                nc.scalar.activation(rb[:, sl], lnd[:, sl], AF.Exp, scale=-1.0)
                p_av = pav.tile([128, 512], f32, tag="pav")
                for c in range(NT):
                    nc.tensor.matmul(p_av, v_nat[:, c, :], expw[:, c, sl], start=(c == 0), stop=(c == NT - 1))
                nc.vector.tensor_mul(attnT[:, sl], p_av, rb[:, sl])
            st["attnT"] = attnT

        def phase3(b, st):
            """gated update tail, pipelined in 512-wide halves; store."""
            x_nat, xT, attnT = st["x_nat"], st["xT"], st["attnT"]
            u = sb.tile([128, N], f32, tag="u")
            gp = sb.tile([128, N], bf16, tag="gp")
            sgy = sb.tile([128, N], f32, tag="sgy")
            sgw = sb.tile([128, N], f32, tag="sgw")
            gate = sb.tile([128, N], f32, tag="gate")
            dlt = sb.tile([128, N], bf16, tag="dlt")
            dlt_nat = sb.tile([128, NT, 128], bf16, tag="dlt_nat")
            o = sb.tile([128, NT, D], f32, tag="o")
            out_r = out_d[b].rearrange("(c p) d -> p c d", p=128)
            H = NT // 2
            for h in range(2):
                sl = slice(h * 512, (h + 1) * 512)
                cs = slice(h * H, (h + 1) * H)

                p_m = ph.tile([128, 512], f32, tag="pwh")
                nc.tensor.matmul(p_m, W["Wo"], attnT[:, sl], start=True, stop=False)
                nc.tensor.matmul(p_m, W["Wo1m"], xT[:, sl], start=False, stop=True)
                copyback(u[:, sl], p_m, "bo_u", nc.vector.tensor_copy)

                p_g = ph.tile([128, 512], f32, tag="pwh")
                nc.tensor.matmul(p_g, W["Wg1"], xT[:, sl], start=True, stop=False)
                nc.tensor.matmul(p_g, W["Wog2"], attnT[:, sl], start=False, stop=True)
                if modes["bo_g"][0] == "zero":
                    nc.vector.tensor_scalar(gp[:, sl], p_g, 0.0, None, op0=OP.max)
                else:
                    nc.scalar.activation(gp[:, sl], p_g, AF.Relu, bias=act_bias("bo_g"))

                p_g3 = ph.tile([128, 512], f32, tag="pwh")
                nc.tensor.matmul(p_g3, W["Wg3"], gp[:, sl], start=True, stop=True)
                nc.scalar.activation(sgy[:, sl], p_g3, AF.Exp, scale=-1.0, bias=act_bias("bg3n"))
                nc.scalar.activation(sgw[:, sl], sgy[:, sl], AF.Ln, bias=1.0)
                nc.scalar.activation(gate[:, sl], sgw[:, sl], AF.Exp, scale=-1.0)

                nc.vector.tensor_mul(dlt[:, sl], u[:, sl], gate[:, sl])
                nc.sync.dma_start_transpose(dlt_nat[:, cs, :], dlt[:, sl])
                nc.vector.tensor_add(o[:, cs, :], dlt_nat[:, cs, :], x_nat[:, cs, :])
                nc.sync.dma_start(out_r[:, cs, :], o[:, cs, :])

        # Software pipeline: emit P3(k-2), P2(k-1), P1(k) per step so each
        # engine's in-order stream interleaves three samples.
        for k in range(BPC + 2):
            if 0 <= k - 2:
                phase3(k - 2, ST[k - 2])
            if 0 <= k - 1 < BPC:
                phase2(ST[k - 1])
            if k < BPC:
                ST[k] = phase1(k)

    # Force Exp and Ln to resolve to the one table set that holds both
    # (natural_log_exp_and_others): contents-only lie to the set chooser,
    # dict order (= act_func_set_id) preserved; the set actually loaded at
    # runtime does contain both functions.
    import concourse.bacc as bacc_mod

    real_get = bacc_mod.get_activation_tables
    target = "natural_log_exp_and_others"

    def patched_get(arch):
        tabs = real_get(arch)
        strip = {AF.Exp, AF.Ln}
        return {
            name: (set(fns) if name == target else set(fns) - strip)
            for name, fns in tabs.items()
        }

    bacc_mod.get_activation_tables = patched_get
    try:
        nc.compile()
    finally:
        bacc_mod.get_activation_tables = real_get
    return nc


def _prep_host(inputs):
    """Host-side: fold weights/biases; returns (f32 inputs, weights bf16, biases)."""
    f32 = np.float32
    g = {k: np.asarray(v, f32) for k, v in inputs.items()}

    Wo1m = g["Wo1"] - np.eye(D, dtype=f32)
    Wog2 = g["Wo"] @ g["Wg2"]                      # msg path folded into gate
    bo_msg = g["bo"] + g["bv"] @ g["Wo"]           # bv folded through Wo
    bo_u = bo_msg + g["bo1"]                       # msg bias + ret bias
    bo_g = bo_msg @ g["Wg2"] + g["bg1"] + g["bg2"]
    bg3n = -g["bg3"]

    wmap = {
        "Wq": g["Wq"], "Wk": g["Wk"], "Wv": g["Wv"], "Wo": g["Wo"],
        "Wo1m": Wo1m, "Wg1": g["Wg1"], "Wog2": Wog2, "Wg3": g["Wg3"],
    }
    bmap = {
        "bq": g["bq"], "bk": g["bk"],
        "bo_u": bo_u, "bo_g": bo_g, "bg3n": bg3n,
    }
    bf16 = ml_dtypes.bfloat16
    wcast = {n: np.ascontiguousarray(w.astype(bf16)) for n, w in wmap.items()}
    return g, wcast, bmap


def _prep_inputs(inputs):
    g, wcast, bmap = _prep_host(inputs)
    modes = {n: _bias_mode(v) for n, v in bmap.items()}
    base = dict(wcast)
    for n, v in bmap.items():
        if modes[n][0] == "ap":
            base[n] = np.ascontiguousarray(v.reshape(D, 1).astype(np.float32))
    x = np.ascontiguousarray(g["x"])
    xbf = np.ascontiguousarray(x.astype(ml_dtypes.bfloat16))
    in_maps = []
    for c in range(NCORES):
        m = dict(base)
        m["x"] = np.ascontiguousarray(x[c * BPC:(c + 1) * BPC])
        m["xbf"] = np.ascontiguousarray(xbf[c * BPC:(c + 1) * BPC])
        in_maps.append(m)
    return in_maps, modes


def kernel(**inputs):
    from concourse.bass_utils import run_bass_kernel_spmd

    in_maps, modes = _prep_inputs(inputs)
    key = tuple(sorted((n, k[0], k[1]) for n, k in modes.items()))
    if _CACHE.get("key") != key:
        _CACHE["nc"] = _build_nc(modes)
        _CACHE["key"] = key
    nc = _CACHE["nc"]

    res = run_bass_kernel_spmd(nc, in_maps, list(range(NCORES)))
    out = np.concatenate([r["out"] for r in res.results], axis=0)
    return out.astype(np.float32)


# revision 49
# speedup vs baseline: 1.6299x; 1.1883x over previous
"""Trainium2 Bass kernel for nn_Net_5488968204310 (gnn_message_passing).

Single-head self-attention (D=128) over N=1024 nodes + gated residual update,
batch B=32, data-parallel across 8 NeuronCores (4 samples per core).

Design notes:
  - "T layout" (features d on partitions, nodes on free dim) for every matmul;
    all eight 128x128 weight matmuls contract over d.
  - x is uploaded twice: fp32 (final residual add) and bf16 (DMA-transposed
    straight into xT; layout transposes run on the DMA xbar, not the PE).
  - QK^T: logitsT chunks [m_chunk(128) x q(1024)] = kT_chunk.T @ qT; exp() on
    the scalar engine directly from PSUM with the 1/sqrt(D) scale folded in.
  - AV keeps v as the stationary operand (few LDWEIGHTS, dense 512-col
    streams): attnT[d,q] = sum_c v_nat[c].T @ expw[c]. The softmax denominator
    is a parallel ones.T @ expw accumulation; 1/denom = exp(-ln(denom)) on the
    scalar engine (DVE reciprocal is ~6 cyc/elem - too slow).
  - sigmoid(z+bg3) = exp(-ln(1+exp(-z-bg3))): every ACT function used (Exp,
    Ln, Relu-free paths) lives in the natural_log_exp_and_others table set,
    enforced by a scoped patch of the table metadata at compile time, so there
    is exactly one ACT_TABLE_LOAD per run.
  - Host folds: Wo1 -> Wo1 - I (so x@(Wo1-I)+msg = ret-x directly),
    Wo@Wg2 (gate path skips msg), bv -> bo terms, bias sums.
"""

import math

import numpy as np
import ml_dtypes

B, N, D = 32, 1024, 128
NCORES = 8
BPC = B // NCORES  # samples per core
NT = N // 128      # node chunks per sample

_CACHE = {}


def _bias_mode(vec):
    """(kind, value) where kind in {'zero', 'uniform', 'ap'}."""
    v = np.asarray(vec, np.float32)
    if not np.any(v):
        return ("zero", 0.0)
    if np.all(v == v.flat[0]):
        return ("uniform", float(v.flat[0]))
    return ("ap", 0.0)


def _build_nc(modes):
    import concourse.bacc as bacc
    import concourse.tile as tile
    from concourse import mybir
    from contextlib import ExitStack

    f32 = mybir.dt.float32
    bf16 = mybir.dt.bfloat16
    f8 = mybir.dt.float8e4
    AF = mybir.ActivationFunctionType
    OP = mybir.AluOpType

    nc = bacc.Bacc("TRN2", target_bir_lowering=False, debug=False)

    x_d = nc.dram_tensor("x", [BPC, N, D], f32, kind="ExternalInput")
    xb_d = nc.dram_tensor("xbf", [BPC, N, D], bf16, kind="ExternalInput")
    out_d = nc.dram_tensor("out", [BPC, N, D], f32, kind="ExternalOutput")
    wnames = ["Wq", "Wk", "Wv", "Wo", "Wo1m", "Wg1", "Wog2", "Wg3"]
    w_d = {n: nc.dram_tensor(n, [D, D], bf16, kind="ExternalInput") for n in wnames}
    b_d = {
        n: nc.dram_tensor(n, [D, 1], f32, kind="ExternalInput")
        for n in modes if modes[n][0] == "ap"
    }

    s = 1.0 / math.sqrt(D)

    with tile.TileContext(nc) as tc, ExitStack() as ctx:
        consts = ctx.enter_context(tc.tile_pool(name="consts", bufs=1))
        sb = ctx.enter_context(tc.tile_pool(name="sb", bufs=2))
        sb3 = ctx.enter_context(tc.tile_pool(name="sb3", bufs=3))
        expp = ctx.enter_context(tc.tile_pool(name="expp", bufs=2))
        pw = ctx.enter_context(tc.tile_pool(name="pw", bufs=2, space="PSUM"))
        ph = ctx.enter_context(tc.tile_pool(name="ph", bufs=2, space="PSUM"))
        pav = ctx.enter_context(tc.tile_pool(name="pav", bufs=1, space="PSUM"))
        pden = ctx.enter_context(tc.tile_pool(name="pden", bufs=1, space="PSUM"))

        W = {}
        for n in wnames:
            t = consts.tile([D, D], bf16, tag=f"w_{n}")
            nc.sync.dma_start(t, w_d[n][:, :])
            W[n] = t
        ones_dr = consts.tile([128, 2, 128], f8, tag="ones_dr")
        nc.vector.memset(ones_dr, 1.0)
        expbias = consts.tile([128, 1], f32, tag="expbias")
        nc.vector.memset(expbias, -2.0)
        BV = {}
        for n in b_d:
            t = consts.tile([D, 1], f32, tag=f"b_{n}")
            nc.sync.dma_start(t, b_d[n][:, :])
            BV[n] = t
        for n, (kind, val) in modes.items():
            if kind == "uniform":
                t = consts.tile([D, 1], f32, tag=f"b_{n}")
                nc.vector.memset(t, val)
                BV[n] = t

        def copyback(dst, src, bname, engine_copy):
            """psum->sbuf copy honoring the bias mode for `bname`."""
            kind, val = modes[bname]
            if kind == "zero":
                engine_copy(dst, src)
            else:
                nc.scalar.activation(dst, src, AF.Identity, bias=BV[bname])

        def act_bias(bname):
            kind, val = modes[bname]
            return 0.0 if kind == "zero" else BV[bname]

        ST = {}

        def phase1(b):
            """loads, q/k/v projections, QK^T + exp."""
            st = {}
            x_nat = sb3.tile([128, NT, D], f32, tag="x_nat")
            nc.sync.dma_start(x_nat, x_d[b].rearrange("(c p) d -> p c d", p=128))
            xT = sb3.tile([128, N], bf16, tag="xT")  # [d, n]
            nc.sync.dma_start_transpose(xT, xb_d[b])
            st["x_nat"], st["xT"] = x_nat, xT

            p_q = pw.tile([128, N], f32, tag="pw")
            nc.tensor.matmul(p_q[:, 0:512], W["Wq"], xT[:, 0:512], start=True, stop=True)
            nc.tensor.matmul(p_q[:, 512:1024], W["Wq"], xT[:, 512:1024], start=True, stop=True)
            qT = sb.tile([128, N], bf16, tag="qT")
            copyback(qT, p_q, "bq", nc.vector.tensor_copy)

            p_k = pw.tile([128, N], f32, tag="pw")
            nc.tensor.matmul(p_k[:, 0:512], W["Wk"], xT[:, 0:512], start=True, stop=True)
            nc.tensor.matmul(p_k[:, 512:1024], W["Wk"], xT[:, 512:1024], start=True, stop=True)
            kT = sb.tile([128, N], bf16, tag="kT")
            copyback(kT, p_k, "bk", nc.vector.tensor_copy)

            p_v = pw.tile([128, N], f32, tag="pw")
            for c in range(NT):
                nc.tensor.matmul(p_v[:, c * 128:(c + 1) * 128], xT[:, c * 128:(c + 1) * 128], W["Wv"], start=True, stop=True)
            v_nat = sb.tile([128, NT, 128], f8, tag="v_nat")
            nc.vector.tensor_copy(v_nat, p_v.rearrange("p (c n) -> p c n", c=NT))
            st["v_nat"] = v_nat

            # exp output in fp8e4m3: bias -2 rescales exp into fp8 range; the
            # uniform factor e^-2 cancels between numerator and denominator.
            expw = expp.tile([128, NT, N], f8, tag="expw")  # [m', c_m, q]
            for c in range(NT):
                p_l = pw.tile([128, N], f32, tag="pw")
                kTc = kT[:, c * 128:(c + 1) * 128]
                nc.tensor.matmul(p_l[:, 0:512], kTc, qT[:, 0:512], start=True, stop=True)
                nc.tensor.matmul(p_l[:, 512:1024], kTc, qT[:, 512:1024], start=True, stop=True)
                nc.scalar.activation(expw[:, c, :], p_l, AF.Exp, scale=s, bias=expbias)
            st["expw"] = expw
            return st

        def phase2(st):
            """denominator + reciprocal + AV + normalize -> attnT."""
            expw, v_nat = st["expw"], st["v_nat"]
            lnd = sb.tile([128, N], f32, tag="lnd")
            rb = sb.tile([128, N], f32, tag="rb")
            attnT = sb.tile([128, N], bf16, tag="attnT")
            for h in range(2):
                sl = slice(h * 512, (h + 1) * 512)
                p_dn = pden.tile([128, 512], f32, tag="pden")
                for c in range(NT // 2):
                    nc.tensor.matmul(
                        p_dn, ones_dr, expw[:, 2 * c:2 * c + 2, sl],
                        start=(c == 0), stop=(c == NT // 2 - 1),
                        perf_mode=mybir.MatmulPerfMode.DoubleRow,
                    )
                nc.scalar.activation(lnd[:, sl], p_dn, AF.Ln)
                nc.scalar.activation(rb[:, sl], lnd[:, sl], AF.Exp, scale=-1.0)
                p_av = pav.tile([128, 512], f32, tag="pav")
                for c in range(NT // 2):
                    nc.tensor.matmul(
                        p_av, v_nat[:, 2 * c:2 * c + 2, :], expw[:, 2 * c:2 * c + 2, sl],
                        start=(c == 0), stop=(c == NT // 2 - 1),
                        perf_mode=mybir.MatmulPerfMode.DoubleRow,
                    )
                nc.vector.tensor_mul(attnT[:, sl], p_av, rb[:, sl])
            st["attnT"] = attnT

        def phase3(b, st):
            """gated update tail, pipelined in 512-wide halves; store."""
            x_nat, xT, attnT = st["x_nat"], st["xT"], st["attnT"]
            u = sb.tile([128, N], f32, tag="u")
            gp = sb.tile([128, N], bf16, tag="gp")
            sgy = sb.tile([128, N], f32, tag="sgy")
            sgw = sb.tile([128, N], f32, tag="sgw")
            gate = sb.tile([128, N], f32, tag="gate")
            dlt = sb.tile([128, N], bf16, tag="dlt")
            dlt_nat = sb.tile([128, NT, 128], bf16, tag="dlt_nat")
            o = sb.tile([128, NT, D], f32, tag="o")
            out_r = out_d[b].rearrange("(c p) d -> p c d", p=128)
            H = NT // 2
            for h in range(2):
                sl = slice(h * 512, (h + 1) * 512)
                cs = slice(h * H, (h + 1) * H)

                p_m = ph.tile([128, 512], f32, tag="pwh")
                nc.tensor.matmul(p_m, W["Wo"], attnT[:, sl], start=True, stop=False)
                nc.tensor.matmul(p_m, W["Wo1m"], xT[:, sl], start=False, stop=True)
                copyback(u[:, sl], p_m, "bo_u", nc.vector.tensor_copy)

                p_g = ph.tile([128, 512], f32, tag="pwh")
                nc.tensor.matmul(p_g, W["Wg1"], xT[:, sl], start=True, stop=False)
                nc.tensor.matmul(p_g, W["Wog2"], attnT[:, sl], start=False, stop=True)
                if modes["bo_g"][0] == "zero":
                    nc.vector.tensor_scalar(gp[:, sl], p_g, 0.0, None, op0=OP.max)
                else:
                    nc.scalar.activation(gp[:, sl], p_g, AF.Relu, bias=act_bias("bo_g"))

                p_g3 = ph.tile([128, 512], f32, tag="pwh")
                nc.tensor.matmul(p_g3, W["Wg3"], gp[:, sl], start=True, stop=True)
                nc.scalar.activation(sgy[:, sl], p_g3, AF.Exp, scale=-1.0, bias=act_bias("bg3n"))
                nc.scalar.activation(sgw[:, sl], sgy[:, sl], AF.Ln, bias=1.0)
                nc.scalar.activation(gate[:, sl], sgw[:, sl], AF.Exp, scale=-1.0)

                nc.vector.tensor_mul(dlt[:, sl], u[:, sl], gate[:, sl])
                nc.sync.dma_start_transpose(dlt_nat[:, cs, :], dlt[:, sl])
                nc.vector.tensor_add(o[:, cs, :], dlt_nat[:, cs, :], x_nat[:, cs, :])
                nc.sync.dma_start(out_r[:, cs, :], o[:, cs, :])

        # Software pipeline: emit P3(k-2), P2(k-1), P1(k) per step so each
        # engine's in-order stream interleaves three samples.
        for k in range(BPC + 2):
            if 0 <= k - 2:
                phase3(k - 2, ST[k - 2])
            if 0 <= k - 1 < BPC:
                phase2(ST[k - 1])
            if k < BPC:
                ST[k] = phase1(k)

    # Force Exp and Ln to resolve to the one table set that holds both
    # (natural_log_exp_and_others): contents-only lie to the set chooser,
    # dict order (= act_func_set_id) preserved; the set actually loaded at
    # runtime does contain both functions.
    import concourse.bacc as bacc_mod

    real_get = bacc_mod.get_activation_tables
    target = "natural_log_exp_and_others"

    def patched_get(arch):
        tabs = real_get(arch)
        strip = {AF.Exp, AF.Ln}
        return {
            name: (set(fns) if name == target else set(fns) - strip)
            for name, fns in tabs.items()
        }

    bacc_mod.get_activation_tables = patched_get
    try:
        nc.compile()
    finally:
        bacc_mod.get_activation_tables = real_get
    return nc


def _prep_host(inputs):
    """Host-side: fold weights/biases; returns (f32 inputs, weights bf16, biases)."""
    f32 = np.float32
    g = {k: np.asarray(v, f32) for k, v in inputs.items()}

    Wo1m = g["Wo1"] - np.eye(D, dtype=f32)
    Wog2 = g["Wo"] @ g["Wg2"]                      # msg path folded into gate
    bo_msg = g["bo"] + g["bv"] @ g["Wo"]           # bv folded through Wo
    bo_u = bo_msg + g["bo1"]                       # msg bias + ret bias
    bo_g = bo_msg @ g["Wg2"] + g["bg1"] + g["bg2"]
    bg3n = -g["bg3"]

    wmap = {
        "Wq": g["Wq"], "Wk": g["Wk"], "Wv": g["Wv"], "Wo": g["Wo"],
        "Wo1m": Wo1m, "Wg1": g["Wg1"], "Wog2": Wog2, "Wg3": g["Wg3"],
    }
    bmap = {
        "bq": g["bq"], "bk": g["bk"],
        "bo_u": bo_u, "bo_g": bo_g, "bg3n": bg3n,
    }
    bf16 = ml_dtypes.bfloat16
    wcast = {n: np.ascontiguousarray(w.astype(bf16)) for n, w in wmap.items()}
    return g, wcast, bmap


def _prep_inputs(inputs):
    g, wcast, bmap = _prep_host(inputs)
    modes = {n: _bias_mode(v) for n, v in bmap.items()}
    base = dict(wcast)
    for n, v in bmap.items():
        if modes[n][0] == "ap":
            base[n] = np.ascontiguousarray(v.reshape(D, 1).astype(np.float32))
    x = np.ascontiguousarray(g["x"])
    xbf = np.ascontiguousarray(x.astype(ml_dtypes.bfloat16))
    in_maps = []
    for c in range(NCORES):
        m = dict(base)
        m["x"] = np.ascontiguousarray(x[c * BPC:(c + 1) * BPC])
        m["xbf"] = np.ascontiguousarray(xbf[c * BPC:(c + 1) * BPC])
        in_maps.append(m)
    return in_maps, modes


def kernel(**inputs):
    from concourse.bass_utils import run_bass_kernel_spmd

    in_maps, modes = _prep_inputs(inputs)
    key = tuple(sorted((n, k[0], k[1]) for n, k in modes.items()))
    if _CACHE.get("key") != key:
        _CACHE["nc"] = _build_nc(modes)
        _CACHE["key"] = key
    nc = _CACHE["nc"]

    res = run_bass_kernel_spmd(nc, in_maps, list(range(NCORES)))
    out = np.concatenate([r["out"] for r in res.results], axis=0)
    return out.astype(np.float32)


# revision 55
# speedup vs baseline: 1.6531x; 1.0143x over previous
"""Trainium2 Bass kernel for nn_Net_5488968204310 (gnn_message_passing).

Single-head self-attention (D=128) over N=1024 nodes + gated residual update,
batch B=32, data-parallel across 8 NeuronCores (4 samples per core).

Design notes:
  - "T layout" (features d on partitions, nodes on free dim) for every matmul;
    all eight 128x128 weight matmuls contract over d.
  - x is uploaded twice: fp32 (final residual add) and bf16 (DMA-transposed
    straight into xT; layout transposes run on the DMA xbar, not the PE).
  - QK^T: logitsT chunks [m_chunk(128) x q(1024)] = kT_chunk.T @ qT; exp() on
    the scalar engine directly from PSUM with the 1/sqrt(D) scale folded in.
  - AV keeps v as the stationary operand (few LDWEIGHTS, dense 512-col
    streams): attnT[d,q] = sum_c v_nat[c].T @ expw[c]. The softmax denominator
    is a parallel ones.T @ expw accumulation; 1/denom = exp(-ln(denom)) on the
    scalar engine (DVE reciprocal is ~6 cyc/elem - too slow).
  - sigmoid(z+bg3) = exp(-ln(1+exp(-z-bg3))): every ACT function used (Exp,
    Ln, Relu-free paths) lives in the natural_log_exp_and_others table set,
    enforced by a scoped patch of the table metadata at compile time, so there
    is exactly one ACT_TABLE_LOAD per run.
  - Host folds: Wo1 -> Wo1 - I (so x@(Wo1-I)+msg = ret-x directly),
    Wo@Wg2 (gate path skips msg), bv -> bo terms, bias sums.
"""

import math

import numpy as np
import ml_dtypes

B, N, D = 32, 1024, 128
NCORES = 8
BPC = B // NCORES  # samples per core
NT = N // 128      # node chunks per sample

_CACHE = {}


def _bias_mode(vec):
    """(kind, value) where kind in {'zero', 'uniform', 'ap'}."""
    v = np.asarray(vec, np.float32)
    if not np.any(v):
        return ("zero", 0.0)
    if np.all(v == v.flat[0]):
        return ("uniform", float(v.flat[0]))
    return ("ap", 0.0)


def _build_nc(modes):
    import concourse.bacc as bacc
    import concourse.tile as tile
    from concourse import mybir
    from contextlib import ExitStack

    f32 = mybir.dt.float32
    bf16 = mybir.dt.bfloat16
    f8 = mybir.dt.float8e4
    AF = mybir.ActivationFunctionType
    OP = mybir.AluOpType

    nc = bacc.Bacc("TRN2", target_bir_lowering=False, debug=False)

    x_d = nc.dram_tensor("x", [BPC, N, D], f32, kind="ExternalInput")
    xb_d = nc.dram_tensor("xbf", [BPC, N, D], bf16, kind="ExternalInput")
    out_d = nc.dram_tensor("out", [BPC, N, D], f32, kind="ExternalOutput")
    wnames = ["Wq", "Wk", "Wv", "Wo", "Wo1m", "Wg1", "Wog2", "Wg3"]
    w_d = {n: nc.dram_tensor(n, [D, D], bf16, kind="ExternalInput") for n in wnames}
    b_d = {
        n: nc.dram_tensor(n, [D, 1], f32, kind="ExternalInput")
        for n in modes if modes[n][0] == "ap"
    }

    s = 1.0 / math.sqrt(D)

    with tile.TileContext(nc) as tc, ExitStack() as ctx:
        consts = ctx.enter_context(tc.tile_pool(name="consts", bufs=1))
        sb = ctx.enter_context(tc.tile_pool(name="sb", bufs=2))
        sb3 = ctx.enter_context(tc.tile_pool(name="sb3", bufs=3))
        expp = ctx.enter_context(tc.tile_pool(name="expp", bufs=2))
        pw = ctx.enter_context(tc.tile_pool(name="pw", bufs=2, space="PSUM"))
        ph = ctx.enter_context(tc.tile_pool(name="ph", bufs=2, space="PSUM"))
        pav = ctx.enter_context(tc.tile_pool(name="pav", bufs=1, space="PSUM"))
        pden = ctx.enter_context(tc.tile_pool(name="pden", bufs=1, space="PSUM"))

        W = {}
        for n in wnames:
            t = consts.tile([D, D], bf16, tag=f"w_{n}")
            nc.sync.dma_start(t, w_d[n][:, :])
            W[n] = t
        ones_dr = consts.tile([128, 2, 128], f8, tag="ones_dr")
        nc.vector.memset(ones_dr, 1.0)
        expbias = consts.tile([128, 1], f32, tag="expbias")
        nc.vector.memset(expbias, -2.0)
        BV = {}
        for n in b_d:
            t = consts.tile([D, 1], f32, tag=f"b_{n}")
            nc.sync.dma_start(t, b_d[n][:, :])
            BV[n] = t
        for n, (kind, val) in modes.items():
            if kind == "uniform":
                t = consts.tile([D, 1], f32, tag=f"b_{n}")
                nc.vector.memset(t, val)
                BV[n] = t

        def copyback(dst, src, bname, engine_copy):
            """psum->sbuf copy honoring the bias mode for `bname`."""
            kind, val = modes[bname]
            if kind == "zero":
                engine_copy(dst, src)
            else:
                nc.scalar.activation(dst, src, AF.Identity, bias=BV[bname])

        def act_bias(bname):
            kind, val = modes[bname]
            return 0.0 if kind == "zero" else BV[bname]

        ST = {}

        def phase1(b):
            """loads, q/k/v projections, QK^T + exp."""
            st = {}
            x_nat = sb3.tile([128, NT, D], f32, tag="x_nat")
            nc.sync.dma_start(x_nat, x_d[b].rearrange("(c p) d -> p c d", p=128))
            xT = sb3.tile([128, N], bf16, tag="xT")  # [d, n]
            nc.sync.dma_start_transpose(xT, xb_d[b])
            st["x_nat"], st["xT"] = x_nat, xT

            p_q = pw.tile([128, N], f32, tag="pw")
            nc.tensor.matmul(p_q[:, 0:512], W["Wq"], xT[:, 0:512], start=True, stop=True)
            nc.tensor.matmul(p_q[:, 512:1024], W["Wq"], xT[:, 512:1024], start=True, stop=True)
            qT = sb.tile([128, N], bf16, tag="qT")
            copyback(qT, p_q, "bq", nc.vector.tensor_copy)

            p_k = pw.tile([128, N], f32, tag="pw")
            nc.tensor.matmul(p_k[:, 0:512], W["Wk"], xT[:, 0:512], start=True, stop=True)
            nc.tensor.matmul(p_k[:, 512:1024], W["Wk"], xT[:, 512:1024], start=True, stop=True)
            kT = sb.tile([128, N], bf16, tag="kT")
            copyback(kT, p_k, "bk", nc.vector.tensor_copy)

            p_v = pw.tile([128, N], f32, tag="pw")
            for c in range(NT):
                nc.tensor.matmul(p_v[:, c * 128:(c + 1) * 128], xT[:, c * 128:(c + 1) * 128], W["Wv"], start=True, stop=True)
            v_nat = sb.tile([128, NT, 128], f8, tag="v_nat")
            nc.vector.tensor_copy(v_nat, p_v.rearrange("p (c n) -> p c n", c=NT))
            st["v_nat"] = v_nat

            # exp output in fp8e4m3: bias -2 rescales exp into fp8 range; the
            # uniform factor e^-2 cancels between numerator and denominator.
            expw = expp.tile([128, NT, N], f8, tag="expw")  # [m', c_m, q]
            for c in range(NT):
                p_l = pw.tile([128, N], f32, tag="pw")
                kTc = kT[:, c * 128:(c + 1) * 128]
                nc.tensor.matmul(p_l[:, 0:512], kTc, qT[:, 0:512], start=True, stop=True)
                nc.tensor.matmul(p_l[:, 512:1024], kTc, qT[:, 512:1024], start=True, stop=True)
                nc.scalar.activation(expw[:, c, :], p_l, AF.Exp, scale=s, bias=expbias)
            st["expw"] = expw
            return st

        def phase2(st):
            """denominator + reciprocal + AV + normalize -> attnT."""
            expw, v_nat = st["expw"], st["v_nat"]
            lnd = sb.tile([128, N], f32, tag="lnd")
            rb = sb.tile([128, N], f32, tag="rb")
            attnT = sb.tile([128, N], bf16, tag="attnT")
            for h in range(2):
                sl = slice(h * 512, (h + 1) * 512)
                p_dn = pden.tile([128, 512], f32, tag="pden")
                for c in range(NT // 2):
                    nc.tensor.matmul(
                        p_dn, ones_dr, expw[:, 2 * c:2 * c + 2, sl],
                        start=(c == 0), stop=(c == NT // 2 - 1),
                        perf_mode=mybir.MatmulPerfMode.DoubleRow,
                    )
                nc.scalar.activation(lnd[:, sl], p_dn, AF.Ln)
                nc.scalar.activation(rb[:, sl], lnd[:, sl], AF.Exp, scale=-1.0)
                p_av = pav.tile([128, 512], f32, tag="pav")
                for c in range(NT // 2):
                    nc.tensor.matmul(
                        p_av, v_nat[:, 2 * c:2 * c + 2, :], expw[:, 2 * c:2 * c + 2, sl],
                        start=(c == 0), stop=(c == NT // 2 - 1),
                        perf_mode=mybir.MatmulPerfMode.DoubleRow,
                    )
                nc.vector.tensor_mul(attnT[:, sl], p_av, rb[:, sl])
            st["attnT"] = attnT

        def phase3(b, st):
            """gated update tail, pipelined in 512-wide halves; store."""
            x_nat, xT, attnT = st["x_nat"], st["xT"], st["attnT"]
            u = sb.tile([128, N], f32, tag="u")
            gp = sb.tile([128, N], bf16, tag="gp")
            sgy = sb.tile([128, N], f32, tag="sgy")
            gate = sb.tile([128, N], f32, tag="gate")
            dlt = sb.tile([128, N], bf16, tag="dlt")
            dlt_nat = sb.tile([128, NT, 128], bf16, tag="dlt_nat")
            o = sb.tile([128, NT, D], f32, tag="o")
            out_r = out_d[b].rearrange("(c p) d -> p c d", p=128)
            H = NT // 2
            for h in range(2):
                sl = slice(h * 512, (h + 1) * 512)
                cs = slice(h * H, (h + 1) * H)

                p_m = ph.tile([128, 512], f32, tag="pwh")
                nc.tensor.matmul(p_m, W["Wo"], attnT[:, sl], start=True, stop=False)
                nc.tensor.matmul(p_m, W["Wo1m"], xT[:, sl], start=False, stop=True)
                copyback(u[:, sl], p_m, "bo_u", nc.vector.tensor_copy)

                p_g = ph.tile([128, 512], f32, tag="pwh")
                nc.tensor.matmul(p_g, W["Wg1"], xT[:, sl], start=True, stop=False)
                nc.tensor.matmul(p_g, W["Wog2"], attnT[:, sl], start=False, stop=True)
                if modes["bo_g"][0] == "zero":
                    nc.vector.tensor_scalar(gp[:, sl], p_g, 0.0, None, op0=OP.max)
                else:
                    nc.scalar.activation(gp[:, sl], p_g, AF.Relu, bias=act_bias("bo_g"))

                p_g3 = ph.tile([128, 512], f32, tag="pwh")
                nc.tensor.matmul(p_g3, W["Wg3"], gp[:, sl], start=True, stop=True)
                # sigmoid(z+bg3) = exp(-ln(1 + exp(-z-bg3))), all in the
                # natural_log_exp ACT table set
                nc.scalar.activation(sgy[:, sl], p_g3, AF.Exp, scale=-1.0, bias=act_bias("bg3n"))
                sgw = sb.tile([128, 512], f32, tag="sgw")
                nc.scalar.activation(sgw, sgy[:, sl], AF.Ln, bias=1.0)
                nc.scalar.activation(gate[:, sl], sgw, AF.Exp, scale=-1.0)

                nc.vector.tensor_mul(dlt[:, sl], u[:, sl], gate[:, sl])
                nc.sync.dma_start_transpose(dlt_nat[:, cs, :], dlt[:, sl])
                nc.gpsimd.tensor_add(o[:, cs, :], dlt_nat[:, cs, :], x_nat[:, cs, :])
                nc.sync.dma_start(out_r[:, cs, :], o[:, cs, :])

        # Software pipeline: emit P3(k-2), P2(k-1), P1(k) per step so each
        # engine's in-order stream interleaves three samples.
        for k in range(BPC + 2):
            if 0 <= k - 2:
                phase3(k - 2, ST[k - 2])
            if 0 <= k - 1 < BPC:
                phase2(ST[k - 1])
            if k < BPC:
                ST[k] = phase1(k)

    # Force Exp and Ln to resolve to the one table set that holds both
    # (natural_log_exp_and_others): contents-only lie to the set chooser,
    # dict order (= act_func_set_id) preserved; the set actually loaded at
    # runtime does contain both functions.
    import concourse.bacc as bacc_mod

    real_get = bacc_mod.get_activation_tables
    target = "natural_log_exp_and_others"

    def patched_get(arch):
        tabs = real_get(arch)
        strip = {AF.Exp, AF.Ln}
        return {
            name: (set(fns) if name == target else set(fns) - strip)
            for name, fns in tabs.items()
        }

    bacc_mod.get_activation_tables = patched_get
    try:
        nc.compile()
    finally:
        bacc_mod.get_activation_tables = real_get
    return nc


def _prep_host(inputs):
    """Host-side: fold weights/biases; returns (f32 inputs, weights bf16, biases)."""
    f32 = np.float32
    g = {k: np.asarray(v, f32) for k, v in inputs.items()}

    Wo1m = g["Wo1"] - np.eye(D, dtype=f32)
    Wog2 = g["Wo"] @ g["Wg2"]                      # msg path folded into gate
    bo_msg = g["bo"] + g["bv"] @ g["Wo"]           # bv folded through Wo
    bo_u = bo_msg + g["bo1"]                       # msg bias + ret bias
    bo_g = bo_msg @ g["Wg2"] + g["bg1"] + g["bg2"]
    bg3n = -g["bg3"]

    wmap = {
        "Wq": g["Wq"], "Wk": g["Wk"], "Wv": g["Wv"], "Wo": g["Wo"],
        "Wo1m": Wo1m, "Wg1": g["Wg1"], "Wog2": Wog2, "Wg3": g["Wg3"],
    }
    bmap = {
        "bq": g["bq"], "bk": g["bk"],
        "bo_u": bo_u, "bo_g": bo_g, "bg3n": bg3n,
    }
    bf16 = ml_dtypes.bfloat16
    wcast = {n: np.ascontiguousarray(w.astype(bf16)) for n, w in wmap.items()}
    return g, wcast, bmap


def _prep_inputs(inputs):
    g, wcast, bmap = _prep_host(inputs)
    modes = {n: _bias_mode(v) for n, v in bmap.items()}
    base = dict(wcast)
    for n, v in bmap.items():
        if modes[n][0] == "ap":
            base[n] = np.ascontiguousarray(v.reshape(D, 1).astype(np.float32))
    x = np.ascontiguousarray(g["x"])
    xbf = np.ascontiguousarray(x.astype(ml_dtypes.bfloat16))
    in_maps = []
    for c in range(NCORES):
        m = dict(base)
        m["x"] = np.ascontiguousarray(x[c * BPC:(c + 1) * BPC])
        m["xbf"] = np.ascontiguousarray(xbf[c * BPC:(c + 1) * BPC])
        in_maps.append(m)
    return in_maps, modes


def kernel(**inputs):
    from concourse.bass_utils import run_bass_kernel_spmd

    in_maps, modes = _prep_inputs(inputs)
    key = tuple(sorted((n, k[0], k[1]) for n, k in modes.items()))
    if _CACHE.get("key") != key:
        _CACHE["nc"] = _build_nc(modes)
        _CACHE["key"] = key
    nc = _CACHE["nc"]

    res = run_bass_kernel_spmd(nc, in_maps, list(range(NCORES)))
    out = np.concatenate([r["out"] for r in res.results], axis=0)
    return out.astype(np.float32)
